# revision 27
# baseline (speedup 1.0000x reference)
"""Trainium2 Bass kernel for nn_CAM_6949257085456.

Pure data-parallel over batch: 8 cores x 64 samples. v3 redesign:

  - NSB=4 sub-batches of 16 samples (RB=256 rows): branch(b-1) work is
    interleaved through sub-batch b's GEMM stream, so only branch(3)
    (16 samples) drains at the end -- v2 drained 32 samples in a
    latency-bound, HAM-cold 76us tail.
  - Branch stage rebuilt around row-major 8-sample groups:
      * 2 full 128x128 PE transposes per group (v2: 32 tiny ones)
      * pair-packed att matmuls: 2 samples share one [32,512] matmul
        via A/B zero-structured block-diagonal cvf stationaries
      * avff ([t,(bi,s,c)] layout for the W_a term) built by 8 small
        SBUF->SBUF DMAs on the idle GpSimd queue instead of 16 PE
        transposes + DVE copies
  - All DRAM->SBUF traffic pre-tiled on host; one big 2D DMA each.

Host-side algebraic folds (exact in fp32):
  - vis path: X @ W_red.T @ W_enc2.T == X @ (W_enc2 @ W_red).T
  - regressors have no nonlinearity: feats@Wv1.T@Wv2.T == feats @ (Wv2@Wv1).T

Precision: the dominant vis GEMM runs in fp8 e3m4 (x direct, folded weight
pre-scaled by 64, undone in the visT bias-activation). Everything else is
bf16 with fp32 PSUM accumulation. Set KFP8=0 for the all-bf16 fallback.
"""
import sys

if "/opt/trn_rl_repo" not in sys.path:
    sys.path.insert(0, "/opt/trn_rl_repo")

import numpy as np
import ml_dtypes

import concourse.bacc as bacc
import concourse.bass as bass
import concourse.mybir as mybir
import concourse.tile as tile
from concourse import bass_utils

BF16 = mybir.dt.bfloat16
F8E3 = mybir.dt.float8e3
F32 = mybir.dt.float32
AF = mybir.ActivationFunctionType

import os as _os

USE_FP8 = _os.environ.get("KFP8", "1") == "1"
WSCALE = 64.0
XDT = F8E3 if USE_FP8 else BF16

B, T, DA, DVFULL, DH = 512, 16, 512, 25088, 128
NCORES = 8
SCALE = 1.0 / 16.0  # 1/sqrt(256)

_CACHE = {}


class Dims:
    def __init__(self, DV, S, G=14):
        self.DV = DV
        self.KC = DV // 128           # contraction chunks (196)
        self.G = G                    # chunks per supertile
        self.NST = self.KC // G       # supertiles (14)
        assert self.NST * G == self.KC
        self.S = S                    # samples per core (64)
        # uneven sub-batches: small tail sub-batches shrink the final
        # latency-bound branch drain to a single 8-sample group
        self.SBS_LIST = [16, 16, 16, 8, 8]
        assert sum(self.SBS_LIST) == S
        self.NSB = len(self.SBS_LIST)
        self.RB_LIST = [s * T for s in self.SBS_LIST]
        self.OFF_LIST = [sum(self.RB_LIST[:b]) for b in range(self.NSB)]
        self.RB0 = self.RB_LIST[0]
        self.R = S * T                # rows per core (1024)


def build_graph(tc, io, D: Dims):
    nc = tc.nc
    from contextlib import ExitStack

    with ExitStack() as stack:
        ec = stack.enter_context
        cpool = ec(tc.tile_pool(name="const", bufs=1))
        wpool = ec(tc.tile_pool(name="wred", bufs=1))
        xpool = ec(tc.tile_pool(name="xin", bufs=6))
        actpool = ec(tc.tile_pool(name="acts", bufs=4))
        avfrpool = ec(tc.tile_pool(name="avfr", bufs=3))
        cvfzpool = ec(tc.tile_pool(name="cvfz", bufs=3))
        att4pool = ec(tc.tile_pool(name="att4", bufs=6))
        avffpool = ec(tc.tile_pool(name="avff", bufs=3))
        htsbpool = ec(tc.tile_pool(name="htsb", bufs=8))
        outsbpool = ec(tc.tile_pool(name="outsb", bufs=4))
        finpool = ec(tc.tile_pool(name="fin", bufs=2))
        # PSUM (8 banks of 2KB/partition; each pool buf = 1 bank):
        encpool = ec(tc.tile_pool(name="enc_ps", bufs=1, space="PSUM"))    # 1 bank
        trpool = ec(tc.tile_pool(name="tr_ps", bufs=1, space="PSUM"))      # 1 bank
        # cvf + att share one 3-buf pool: cvf tiles drain fast (DVE copy),
        # leaving 3 rotating banks for the att matmul->tanh pipeline
        attpool = ec(tc.tile_pool(name="att_ps", bufs=3, space="PSUM"))    # 3 banks
        cvfpool = attpool
        htpool = ec(tc.tile_pool(name="ht_ps", bufs=2, space="PSUM"))      # 2 banks
        outpool = ec(tc.tile_pool(name="out_ps", bufs=1, space="PSUM"))    # 1 bank

        # ---- prologue: the very first DMAs feed the vis stream's first
        # supertile so the PE starts as early as possible ----
        wt_big = wpool.tile([128, D.NST, D.G, 128], XDT, tag="wt", name="wt")
        xg00 = xpool.tile([128, D.G, D.RB_LIST[0]], XDT, tag="xg", name="xg0_0")
        gh0 = D.G // 2
        nc.sync.dma_start(xg00[:, 0:gh0, :], io["xg0"][0][:, 0:gh0, :])
        nc.sync.dma_start(wt_big[:, 0, 0:gh0, :], io["wt"][0][:, 0:gh0, :])

        f1_sb = cpool.tile([128, 4, D.R], BF16, name="f1sb")
        for c in range(4):
            nc.sync.dma_start(f1_sb[:, c, 0 : D.RB0], io["f1t"][:, c, 0 : D.RB0])
        wenc1_sb = cpool.tile([128, 4, DH], BF16, name="wenc1")
        nc.sync.dma_start(wenc1_sb[:], io["wenc1"])
        b1_sb = cpool.tile([DH, 1], F32, name="b1sb")
        nc.sync.dma_start(b1_sb[:], io["b1"])
        b2_sb = cpool.tile([DH, 1], F32, name="b2sb")
        nc.sync.dma_start(b2_sb[:], io["b2"])

        ident_sb = cpool.tile([128, 128], BF16, name="ident")
        wblk4_sb = cpool.tile([128, 4, 128], BF16, name="wblk4")
        wa_sb = cpool.tile([16, 32], BF16, name="wasb")
        wca_sb = cpool.tile([128, 2, 32], BF16, name="wcasb")
        wh_sb = cpool.tile([32, 32], BF16, name="whsb")
        wrega_sb = cpool.tile([128, 2], BF16, name="wrega")
        wregv_sb = cpool.tile([128, 2], BF16, name="wregv")
        creg_sb = cpool.tile([2, 1], F32, name="cregsb")

        def issue_branch_consts():
            # scalar queue (also HWDGE): keeps the Sync queue free for the
            # x-stream supertile issues during the DMA-bound first stream
            nc.scalar.dma_start(ident_sb[:], io["ident"])
            nc.scalar.dma_start(wblk4_sb[:], io["wblk4"])
            nc.scalar.dma_start(wa_sb[:], io["waT"])
            nc.scalar.dma_start(wca_sb[:], io["wcaT"])
            nc.scalar.dma_start(wh_sb[:], io["whT"])
            nc.scalar.dma_start(wrega_sb[:], io["wreg_a"])
            nc.scalar.dma_start(wregv_sb[:], io["wreg_v"])
            nc.scalar.dma_start(creg_sb[:], io["creg"])

        def emit_front(ctx, grp):
            """transposes -> avfR; avff DMAs; cvfZ blockdiag matmuls."""
            sb = ctx["b"]
            audT, visT = ctx["audT"], ctx["visT"]
            trt = trpool.tile([128, 256], BF16, tag="tr", name=f"tr{sb}_{grp}")
            nc.tensor.transpose(
                trt[:, 0:128], audT[:, 128 * grp : 128 * grp + 128], ident_sb[:]
            )
            nc.tensor.transpose(
                trt[:, 128:256], visT[:, 128 * grp : 128 * grp + 128], ident_sb[:]
            )
            avfR = avfrpool.tile([128, 256], BF16, tag="avfr", name=f"avfr{sb}_{grp}")
            nc.vector.tensor_copy(avfR[:], trt[:])
            avff = avffpool.tile([16, 2, 8, 128], BF16, tag="avff", name=f"aff{sb}_{grp}")
            for s in range(8):
                # partition-base-shifting gather: only DMA engines may read
                # at non-32-aligned partition bases
                nc.gpsimd.dma_start(
                    avff[:, :, s, :],
                    avfR[16 * s : 16 * s + 16, :].rearrange(
                        "p (bi c) -> p bi c", bi=2
                    ),
                )
            cvfps = cvfpool.tile([128, 512], F32, tag="att", name=f"cvp{sb}_{grp}")
            for i, half in [(0, 0), (1, 1), (2, 0), (3, 1)]:
                nc.tensor.matmul(
                    cvfps[:, 128 * i : 128 * i + 128],
                    wblk4_sb[:, i, :],
                    avfR[:, 128 * half : 128 * half + 128],
                    start=True,
                    stop=True,
                )
            cvfZ = cvfzpool.tile([128, 512], BF16, tag="cvfz", name=f"cvz{sb}_{grp}")
            nc.vector.tensor_copy(cvfZ[:], cvfps[:])
            ctx["avfR"][grp] = avfR
            ctx["avff"][grp] = avff
            ctx["cvfZ"][grp] = cvfZ
            ctx["att4"][grp] = {}

        def emit_att(ctx, grp, m):
            """pair m (samples 2m, 2m+1 of group): att matmuls + tanh."""
            sb = ctx["b"]
            avfR, cvfZ = ctx["avfR"][grp], ctx["cvfZ"][grp]
            if m == 0:
                for jh in range(2):
                    ctx["att4"][grp][jh] = att4pool.tile(
                        [128, 4, 512], BF16, tag="att4", name=f"a4_{sb}_{grp}_{jh}"
                    )
            for jh in range(2):
                aps = attpool.tile([128, 512], F32, tag="att", name=f"ap{sb}_{grp}{m}{jh}")
                nc.tensor.matmul(
                    aps[:],
                    avfR[32 * m : 32 * m + 32, 128 * jh : 128 * jh + 128],
                    cvfZ[32 * m : 32 * m + 32, :],
                    start=True,
                    stop=True,
                    tile_position=(32 * m, 0),
                )
                nc.scalar.activation(
                    ctx["att4"][grp][jh][:, m, :], aps[:], AF.Tanh, scale=SCALE
                )

        def emit_ht(ctx, grp, g):
            """H = relu(Wca@att + Wa@fts) for 4 samples (subgroup g)."""
            sb = ctx["b"]
            avff = ctx["avff"][grp]
            for bi in range(2):
                htps = htpool.tile([32, 512], F32, tag="ht", name=f"ht{sb}_{grp}{g}{bi}")
                nc.tensor.matmul(
                    htps[:],
                    wa_sb[:],
                    avff[:, bi, 4 * g : 4 * g + 4, :],
                    start=True,
                    stop=False,
                )
                for jh in range(2):
                    v = ctx["att4"][grp][jh][:].rearrange(
                        "p m (s2 bi c) -> p m s2 bi c", s2=2, bi=2
                    )
                    nc.tensor.matmul(
                        htps[:],
                        wca_sb[:, jh, :],
                        v[:, 2 * g : 2 * g + 2, :, bi, :],
                        start=False,
                        stop=(jh == 1),
                    )
                htsb = htsbpool.tile(
                    [32, 512], BF16, tag="htsb", name=f"hs{sb}_{grp}{g}{bi}"
                )
                nc.vector.tensor_relu(htsb[:], htps[:])
                ctx["htsb"][(grp, g, bi)] = htsb

        def emit_out(ctx, grp, g):
            """out = Wh@H into outp PSUM."""
            outp = ctx["outp"][grp]
            for bi in range(2):
                htsb = ctx["htsb"].pop((grp, g, bi))
                for q in range(4):
                    c0 = 128 * g + 64 * bi + 16 * q
                    nc.tensor.matmul(
                        outp[:, c0 : c0 + 16],
                        htsb[0:32, 128 * q : 128 * q + 128],
                        wh_sb[0:32, 16 * bi : 16 * bi + 16],
                        start=True,
                        stop=True,
                    )

        def emit_adds(ctx, grp):
            """residual adds into outa/outv (bf16 SBUF)."""
            outp = ctx["outp"][grp]
            opv = outp[:].rearrange("p (g bi q t) -> p g bi q t", g=2, bi=2, q=4)
            oa = ctx["outa"][:, 128 * grp : 128 * grp + 128].rearrange(
                "p (g q t) -> p g q t", g=2, q=4
            )
            ov = ctx["outv"][:, 128 * grp : 128 * grp + 128].rearrange(
                "p (g q t) -> p g q t", g=2, q=4
            )
            aT = ctx["audT"][:, 128 * grp : 128 * grp + 128].rearrange(
                "p (g q t) -> p g q t", g=2, q=4
            )
            vT = ctx["visT"][:, 128 * grp : 128 * grp + 128].rearrange(
                "p (g q t) -> p g q t", g=2, q=4
            )
            nc.vector.tensor_add(oa, opv[:, :, 0, :, :], aT)
            nc.vector.tensor_add(ov, opv[:, :, 1, :, :], vT)

        def emit_reg(ctx):
            sb = ctx["b"]
            rb, off = ctx["RB"], ctx["OFF"]
            regps = trpool.tile([2, 256], F32, tag="tr", name=f"reg{sb}")
            nc.tensor.matmul(
                regps[:, 0:rb], wrega_sb[:], ctx["outa"][:], start=True, stop=False
            )
            nc.tensor.matmul(
                regps[:, 0:rb], wregv_sb[:], ctx["outv"][:], start=False, stop=True
            )
            fin = finpool.tile([2, 256], F32, tag="fin", name=f"fin{sb}")
            nc.scalar.activation(fin[:, 0:rb], regps[:, 0:rb], AF.Identity, bias=creg_sb[:])
            nc.sync.dma_start(io["vouts"][off : off + rb], fin[0:1, 0:rb])
            nc.sync.dma_start(io["aouts"][off : off + rb], fin[1:2, 0:rb])

        def make_units(ctx, ngrp):
            # fronts of all groups first: their transpose->copy->avff chains
            # get maximum lead time before the dependent att/ht work
            units = []
            for grp in range(ngrp):
                units.append(lambda grp=grp: emit_front(ctx, grp))
            for grp in range(ngrp):
                units.append(lambda grp=grp: emit_att(ctx, grp, 0))
                units.append(lambda grp=grp: emit_att(ctx, grp, 1))
                units.append(lambda grp=grp: emit_ht(ctx, grp, 0))
                units.append(lambda grp=grp: emit_att(ctx, grp, 2))
                units.append(lambda grp=grp: emit_att(ctx, grp, 3))
                units.append(lambda grp=grp: emit_ht(ctx, grp, 1))
                units.append(lambda grp=grp: emit_out(ctx, grp, 0))
                units.append(lambda grp=grp: emit_out(ctx, grp, 1))
                units.append(lambda grp=grp: emit_adds(ctx, grp))
            units.append(lambda: emit_reg(ctx))
            return units

        prev_units = []
        for b in range(D.NSB):
            RB = D.RB_LIST[b]
            OFF = D.OFF_LIST[b]
            NGRP = D.SBS_LIST[b] // 8
            ctx = {"b": b, "RB": RB, "OFF": OFF, "avfR": {}, "avff": {},
                   "cvfZ": {}, "att4": {}, "htsb": {}}
            # one PSUM bank holds both encoder accumulators for this sub-batch
            encps = encpool.tile([128, 512], F32, tag="enc", name=f"encps{b}")
            audps = encps[:, 0:RB]
            visps = encps[:, 256 : 256 + RB]
            for c in range(4):
                nc.tensor.matmul(
                    audps,
                    wenc1_sb[:, c, :],
                    f1_sb[:, c, OFF : OFF + RB],
                    start=(c == 0),
                    stop=(c == 3),
                )
            audT = actpool.tile([128, 256], BF16, tag="act", name=f"audT{b}")
            nc.scalar.activation(audT[:, 0:RB], audps, AF.Identity, bias=b1_sb[:])
            ctx["audT"] = audT

            # vis stream, interleaving branch work of sub-batch b-1
            emitted = 0
            prefetch = {}
            for i in range(D.NST):
                xg_dram = io[f"xg{b}"][i]
                if b == 0 and i == 0:
                    xg = xg00
                    gh = D.G // 2
                    nc.sync.dma_start(xg[:, gh:, :], xg_dram[:, gh:, :])
                    nc.sync.dma_start(wt_big[:, 0, gh:, :], io["wt"][0][:, gh:, :])
                elif i in prefetch:
                    xg = prefetch.pop(i)
                else:
                    xg = xpool.tile([128, D.G, RB], XDT, tag="xg", name=f"xg{b}_{i}")
                    nc.sync.dma_start(xg[:], xg_dram)
                    if b == 0 and i % 2 == 1 and i + 1 < D.NST:
                        # pair up stream-0 supertile issues: this stream is
                        # DMA-issue-rate-bound on the sync queue
                        xgn = xpool.tile(
                            [128, D.G, RB], XDT, tag="xg", name=f"xg{b}_{i + 1}"
                        )
                        nc.sync.dma_start(xgn[:], io[f"xg{b}"][i + 1])
                        prefetch[i + 1] = xgn
                if b == 0:
                    if i == 1:
                        nc.sync.dma_start(
                            wt_big[:, 1:4], io["wt"][1:4].rearrange("a p g c -> p a g c")
                        )
                    elif i == 3:
                        nc.sync.dma_start(
                            wt_big[:, 4:9], io["wt"][4:9].rearrange("a p g c -> p a g c")
                        )
                    elif i == 6:
                        nc.sync.dma_start(
                            wt_big[:, 9:14],
                            io["wt"][9:14].rearrange("a p g c -> p a g c"),
                        )
                    elif i == 10:
                        issue_branch_consts()
                    elif i == 12:
                        nc.scalar.dma_start(
                            f1_sb[:, :, D.RB0 :], io["f1t"][:, :, D.RB0 :]
                        )
                for j in range(D.G):
                    k = D.G * i + j
                    nc.tensor.matmul(
                        visps,
                        wt_big[:, i, j, :],
                        xg[:, j, :],
                        start=(k == 0),
                        stop=(k == D.KC - 1),
                    )
                target = (i + 1) * len(prev_units) // D.NST
                while emitted < target:
                    prev_units[emitted]()
                    emitted += 1
            while emitted < len(prev_units):
                prev_units[emitted]()
                emitted += 1
            visT = actpool.tile([128, 256], BF16, tag="act", name=f"visT{b}")
            nc.scalar.activation(
                visT[:, 0:RB],
                visps,
                AF.Identity,
                bias=b2_sb[:],
                scale=(1.0 / WSCALE) if USE_FP8 else 1.0,
            )
            ctx["visT"] = visT
            outa = outsbpool.tile([128, RB], BF16, tag="outsb", name=f"oa{b}")
            outv = outsbpool.tile([128, RB], BF16, tag="outsb", name=f"ov{b}")
            ctx["outa"], ctx["outv"] = outa, outv
            ctx["outp"] = {}
            for grp in range(NGRP):
                ctx["outp"][grp] = outpool.tile(
                    [128, 256], F32, tag="outp", name=f"op{b}_{grp}"
                )
            prev_units = make_units(ctx, NGRP)

        for u in prev_units:
            u()


def _build(D: Dims):
    nc = bacc.Bacc("TRN2", target_bir_lowering=False, debug=False)

    io = {}
    for b in range(D.NSB):
        io[f"xg{b}"] = nc.dram_tensor(
            f"xg{b}", [D.NST, 128, D.G, D.RB_LIST[b]], XDT, kind="ExternalInput"
        ).ap()
    io["wt"] = nc.dram_tensor(
        "wt", [D.NST, 128, D.G, 128], XDT, kind="ExternalInput"
    ).ap()
    io["f1t"] = nc.dram_tensor("f1t", [128, 4, D.R], BF16, kind="ExternalInput").ap()
    io["wenc1"] = nc.dram_tensor("wenc1", [128, 4, DH], BF16, kind="ExternalInput").ap()
    for name, shape, dt in [
        ("b1", [DH, 1], F32),
        ("b2", [DH, 1], F32),
        ("wblk4", [128, 4, 128], BF16),
        ("waT", [16, 32], BF16),
        ("wcaT", [128, 2, 32], BF16),
        ("whT", [32, 32], BF16),
        ("wreg_a", [128, 2], BF16),
        ("wreg_v", [128, 2], BF16),
        ("creg", [2, 1], F32),
        ("ident", [128, 128], BF16),
    ]:
        io[name] = nc.dram_tensor(name, shape, dt, kind="ExternalInput").ap()
    io["vouts"] = nc.dram_tensor("vouts", [D.R], F32, kind="ExternalOutput").ap()
    io["aouts"] = nc.dram_tensor("aouts", [D.R], F32, kind="ExternalOutput").ap()

    with tile.TileContext(nc) as tc:
        build_graph(tc, io, D)

    nc.compile()
    return nc


def prep_shared(inputs, D: Dims):
    f32 = np.float32
    bf = ml_dtypes.bfloat16
    W_enc1 = np.asarray(inputs["W_enc1"], f32)
    W_enc2 = np.asarray(inputs["W_enc2"], f32)
    W_red = np.asarray(inputs["W_red"], f32)
    W2r = W_enc2 @ W_red                                    # [128, DV]
    b2v = W_enc2 @ np.asarray(inputs["b_red"], f32) + np.asarray(inputs["b_enc2"], f32)
    wv = (np.asarray(inputs["Wv2"], f32) @ np.asarray(inputs["Wv1"], f32))[0]
    cv = float((np.asarray(inputs["Wv2"], f32) @ np.asarray(inputs["bv1"], f32)
                + np.asarray(inputs["bv2"], f32))[0])
    wa = (np.asarray(inputs["Wa2"], f32) @ np.asarray(inputs["Wa1"], f32))[0]
    ca = float((np.asarray(inputs["Wa2"], f32) @ np.asarray(inputs["ba1"], f32)
                + np.asarray(inputs["ba2"], f32))[0])

    def mk_wblk(W_aff, odd):
        # lhsT[16s+t', 32m+16*odd+t] = W_aff[t, t'] for s = 2m+odd
        M = np.zeros((128, 128), f32)
        Wt = np.asarray(W_aff, f32).T  # [t', t]
        for s in range(odd, 8, 2):
            m = s // 2
            base = 32 * m + 16 * odd
            M[16 * s : 16 * s + 16, base : base + 16] = Wt
        return M

    wblk4 = np.stack(
        [
            mk_wblk(inputs["W_affa"], 0),
            mk_wblk(inputs["W_affv"], 0),
            mk_wblk(inputs["W_affa"], 1),
            mk_wblk(inputs["W_affv"], 1),
        ],
        axis=1,
    )  # [128, 4, 128]

    wt = (
        np.ascontiguousarray(W2r.T)
        .reshape(D.NST, D.G, 128, 128)
        .transpose(0, 2, 1, 3)
    )
    if USE_FP8:
        wt = wt * WSCALE
    wenc1t = W_enc1.T.reshape(4, 128, DH).transpose(1, 0, 2)

    wh = np.zeros((32, 32), f32)
    wh[:, 0:16] = np.asarray(inputs["W_ha"], f32).T
    wh[:, 16:32] = np.asarray(inputs["W_hv"], f32).T

    shared = {
        "wt": np.ascontiguousarray(wt).astype(
            ml_dtypes.float8_e3m4 if USE_FP8 else bf
        ),
        "wenc1": np.ascontiguousarray(wenc1t).astype(bf),
        "b1": np.asarray(inputs["b_enc1"], f32).reshape(DH, 1),
        "b2": b2v.reshape(DH, 1),
        "wblk4": np.ascontiguousarray(wblk4).astype(bf),
        "waT": np.asarray(inputs["W_a"], f32).T.astype(bf),
        "wcaT": np.asarray(inputs["W_ca"], f32)
        .T.reshape(2, 128, 32)
        .transpose(1, 0, 2)
        .astype(bf)
        .copy(),
        "whT": wh.astype(bf),
        "wreg_a": np.stack([wv[:128], wa[:128]], 1).astype(bf),
        "wreg_v": np.stack([wv[128:], wa[128:]], 1).astype(bf),
        "creg": np.array([[cv], [ca]], f32),
        "ident": np.eye(128, dtype=f32).astype(bf),
    }
    return shared


def prep_core(f1_core, f2_core, D: Dims):
    """Per-core activation tiling. f1_core [R, DA], f2_core [R, DV] fp32."""
    bf = ml_dtypes.bfloat16
    xdt = ml_dtypes.float8_e3m4 if USE_FP8 else bf
    out = {}
    for b in range(D.NSB):
        rb, off = D.RB_LIST[b], D.OFF_LIST[b]
        # xg_b[i, p, j, r] = f2[off + r, (G*i+j)*128 + p]
        xg = (
            f2_core[off : off + rb]
            .reshape(rb, D.NST, D.G, 128)
            .transpose(1, 3, 2, 0)
        )
        out[f"xg{b}"] = np.ascontiguousarray(xg).astype(xdt)
    # f1t[p, c, r] = f1[r, c*128+p]
    f1t = f1_core.reshape(D.R, 4, 128).transpose(2, 1, 0)
    out["f1t"] = np.ascontiguousarray(f1t).astype(bf)
    return out


def kernel(**inputs):
    D = Dims(DV=DVFULL, S=B // NCORES)
    if "nc" not in _CACHE:
        _CACHE["nc"] = _build(D)
    nc = _CACHE["nc"]

    shared = prep_shared(inputs, D)
    f1 = np.asarray(inputs["f1_norm"], np.float32).reshape(B * T, DA)
    f2 = np.asarray(inputs["f2_norm"], np.float32).reshape(B * T, DVFULL)

    in_maps = []
    for c in range(NCORES):
        rs = slice(c * D.R, (c + 1) * D.R)
        m = dict(shared)
        m.update(prep_core(f1[rs], f2[rs], D))
        in_maps.append(m)

    import os

    res = bass_utils.run_bass_kernel_spmd(
        nc,
        in_maps,
        core_ids=list(range(NCORES)),
        trace=bool(os.environ.get("KERNEL_TRACE")),
    )
    _CACHE["last_results"] = res

    S = B // NCORES
    vouts = np.concatenate(
        [r["vouts"].reshape(S, T) for r in res.results], axis=0
    ).astype(np.float32)
    aouts = np.concatenate(
        [r["aouts"].reshape(S, T) for r in res.results], axis=0
    ).astype(np.float32)
    return vouts, aouts


# revision 28
# speedup vs baseline: 1.0580x; 1.0580x over previous
"""Trainium2 Bass kernel for nn_CAM_6949257085456.

Pure data-parallel over batch: 8 cores x 64 samples. v3 redesign:

  - NSB=4 sub-batches of 16 samples (RB=256 rows): branch(b-1) work is
    interleaved through sub-batch b's GEMM stream, so only branch(3)
    (16 samples) drains at the end -- v2 drained 32 samples in a
    latency-bound, HAM-cold 76us tail.
  - Branch stage rebuilt around row-major 8-sample groups:
      * 2 full 128x128 PE transposes per group (v2: 32 tiny ones)
      * pair-packed att matmuls: 2 samples share one [32,512] matmul
        via A/B zero-structured block-diagonal cvf stationaries
      * avff ([t,(bi,s,c)] layout for the W_a term) built by 8 small
        SBUF->SBUF DMAs on the idle GpSimd queue instead of 16 PE
        transposes + DVE copies
  - All DRAM->SBUF traffic pre-tiled on host; one big 2D DMA each.

Host-side algebraic folds (exact in fp32):
  - vis path: X @ W_red.T @ W_enc2.T == X @ (W_enc2 @ W_red).T
  - regressors have no nonlinearity: feats@Wv1.T@Wv2.T == feats @ (Wv2@Wv1).T

Precision: the dominant vis GEMM runs in fp8 e3m4 (x direct, folded weight
pre-scaled by 64, undone in the visT bias-activation). Everything else is
bf16 with fp32 PSUM accumulation. Set KFP8=0 for the all-bf16 fallback.
"""
import sys

if "/opt/trn_rl_repo" not in sys.path:
    sys.path.insert(0, "/opt/trn_rl_repo")

import numpy as np
import ml_dtypes

import concourse.bacc as bacc
import concourse.bass as bass
import concourse.mybir as mybir
import concourse.tile as tile
from concourse import bass_utils

BF16 = mybir.dt.bfloat16
F8E3 = mybir.dt.float8e3
F32 = mybir.dt.float32
AF = mybir.ActivationFunctionType

import os as _os

USE_FP8 = _os.environ.get("KFP8", "1") == "1"
WSCALE = 64.0
XDT = F8E3 if USE_FP8 else BF16

B, T, DA, DVFULL, DH = 512, 16, 512, 25088, 128
NCORES = 8
SCALE = 1.0 / 16.0  # 1/sqrt(256)

_CACHE = {}


class Dims:
    def __init__(self, DV, S, G=14):
        self.DV = DV
        self.KC = DV // 128           # contraction chunks (196)
        self.G = G                    # chunks per supertile
        self.NST = self.KC // G       # supertiles (14)
        assert self.NST * G == self.KC
        self.S = S                    # samples per core (64)
        # uneven sub-batches: small tail sub-batches shrink the final
        # latency-bound branch drain to a single 8-sample group
        self.SBS_LIST = [16, 16, 16, 8, 8]
        assert sum(self.SBS_LIST) == S
        self.NSB = len(self.SBS_LIST)
        self.RB_LIST = [s * T for s in self.SBS_LIST]
        self.OFF_LIST = [sum(self.RB_LIST[:b]) for b in range(self.NSB)]
        self.RB0 = self.RB_LIST[0]
        self.R = S * T                # rows per core (1024)


def build_graph(tc, io, D: Dims):
    nc = tc.nc
    from contextlib import ExitStack

    with ExitStack() as stack:
        ec = stack.enter_context
        cpool = ec(tc.tile_pool(name="const", bufs=1))
        wpool = ec(tc.tile_pool(name="wred", bufs=D.NST))
        xpool = ec(tc.tile_pool(name="xin", bufs=6))
        actpool = ec(tc.tile_pool(name="acts", bufs=4))
        avfrpool = ec(tc.tile_pool(name="avfr", bufs=3))
        cvfzpool = ec(tc.tile_pool(name="cvfz", bufs=3))
        att4pool = ec(tc.tile_pool(name="att4", bufs=6))
        avffpool = ec(tc.tile_pool(name="avff", bufs=3))
        htsbpool = ec(tc.tile_pool(name="htsb", bufs=8))
        outsbpool = ec(tc.tile_pool(name="outsb", bufs=4))
        finpool = ec(tc.tile_pool(name="fin", bufs=2))
        # PSUM (8 banks of 2KB/partition; each pool buf = 1 bank):
        encpool = ec(tc.tile_pool(name="enc_ps", bufs=1, space="PSUM"))    # 1 bank
        trpool = ec(tc.tile_pool(name="tr_ps", bufs=1, space="PSUM"))      # 1 bank
        # cvf + att share one 3-buf pool: cvf tiles drain fast (DVE copy),
        # leaving 3 rotating banks for the att matmul->tanh pipeline
        attpool = ec(tc.tile_pool(name="att_ps", bufs=3, space="PSUM"))    # 3 banks
        cvfpool = attpool
        htpool = ec(tc.tile_pool(name="ht_ps", bufs=2, space="PSUM"))      # 2 banks
        outpool = ec(tc.tile_pool(name="out_ps", bufs=1, space="PSUM"))    # 1 bank

        # ---- prologue: the very first DMAs feed the vis stream's first
        # supertile so the PE starts as early as possible ----
        wt_tiles = []
        for i in range(D.NST):
            wt = wpool.tile([128, D.G, 128], XDT, tag="wt", name=f"wt{i}")
            wt_tiles.append(wt)
        xg00 = xpool.tile([128, D.G, D.RB_LIST[0]], XDT, tag="xg", name="xg0_0")
        gh0 = D.G // 2
        nc.sync.dma_start(xg00[:, 0:gh0, :], io["xg0"][0][:, 0:gh0, :])
        nc.sync.dma_start(wt_tiles[0][:, 0:gh0, :], io["wt"][0][:, 0:gh0, :])

        f1_sb = cpool.tile([128, 4, D.R], BF16, name="f1sb")
        for c in range(4):
            nc.sync.dma_start(f1_sb[:, c, 0 : D.RB0], io["f1t"][:, c, 0 : D.RB0])
        wenc1_sb = cpool.tile([128, 4, DH], BF16, name="wenc1")
        nc.sync.dma_start(wenc1_sb[:], io["wenc1"])
        b1_sb = cpool.tile([DH, 1], F32, name="b1sb")
        nc.sync.dma_start(b1_sb[:], io["b1"])
        b2_sb = cpool.tile([DH, 1], F32, name="b2sb")
        nc.sync.dma_start(b2_sb[:], io["b2"])

        ident_sb = cpool.tile([128, 128], BF16, name="ident")
        wblk4_sb = cpool.tile([128, 4, 128], BF16, name="wblk4")
        wa_sb = cpool.tile([16, 32], BF16, name="wasb")
        wca_sb = cpool.tile([128, 2, 32], BF16, name="wcasb")
        wh_sb = cpool.tile([32, 32], BF16, name="whsb")
        wrega_sb = cpool.tile([128, 2], BF16, name="wrega")
        wregv_sb = cpool.tile([128, 2], BF16, name="wregv")
        creg_sb = cpool.tile([2, 1], F32, name="cregsb")

        def issue_branch_consts():
            # scalar queue (also HWDGE): keeps the Sync queue free for the
            # x-stream supertile issues during the DMA-bound first stream
            nc.scalar.dma_start(ident_sb[:], io["ident"])
            nc.scalar.dma_start(wblk4_sb[:], io["wblk4"])
            nc.scalar.dma_start(wa_sb[:], io["waT"])
            nc.scalar.dma_start(wca_sb[:], io["wcaT"])
            nc.scalar.dma_start(wh_sb[:], io["whT"])
            nc.scalar.dma_start(wrega_sb[:], io["wreg_a"])
            nc.scalar.dma_start(wregv_sb[:], io["wreg_v"])
            nc.scalar.dma_start(creg_sb[:], io["creg"])

        def emit_front(ctx, grp):
            """transposes -> avfR; avff DMAs; cvfZ blockdiag matmuls."""
            sb = ctx["b"]
            audT, visT = ctx["audT"], ctx["visT"]
            trt = trpool.tile([128, 256], BF16, tag="tr", name=f"tr{sb}_{grp}")
            nc.tensor.transpose(
                trt[:, 0:128], audT[:, 128 * grp : 128 * grp + 128], ident_sb[:]
            )
            nc.tensor.transpose(
                trt[:, 128:256], visT[:, 128 * grp : 128 * grp + 128], ident_sb[:]
            )
            avfR = avfrpool.tile([128, 256], BF16, tag="avfr", name=f"avfr{sb}_{grp}")
            nc.vector.tensor_copy(avfR[:], trt[:])
            avff = avffpool.tile([16, 2, 8, 128], BF16, tag="avff", name=f"aff{sb}_{grp}")
            for s in range(8):
                # partition-base-shifting gather: only DMA engines may read
                # at non-32-aligned partition bases
                nc.gpsimd.dma_start(
                    avff[:, :, s, :],
                    avfR[16 * s : 16 * s + 16, :].rearrange(
                        "p (bi c) -> p bi c", bi=2
                    ),
                )
            cvfps = cvfpool.tile([128, 512], F32, tag="att", name=f"cvp{sb}_{grp}")
            for i, half in [(0, 0), (1, 1), (2, 0), (3, 1)]:
                nc.tensor.matmul(
                    cvfps[:, 128 * i : 128 * i + 128],
                    wblk4_sb[:, i, :],
                    avfR[:, 128 * half : 128 * half + 128],
                    start=True,
                    stop=True,
                )
            cvfZ = cvfzpool.tile([128, 512], BF16, tag="cvfz", name=f"cvz{sb}_{grp}")
            nc.vector.tensor_copy(cvfZ[:], cvfps[:])
            ctx["avfR"][grp] = avfR
            ctx["avff"][grp] = avff
            ctx["cvfZ"][grp] = cvfZ
            ctx["att4"][grp] = {}

        def emit_att(ctx, grp, m):
            """pair m (samples 2m, 2m+1 of group): att matmuls + tanh."""
            sb = ctx["b"]
            avfR, cvfZ = ctx["avfR"][grp], ctx["cvfZ"][grp]
            if m == 0:
                for jh in range(2):
                    ctx["att4"][grp][jh] = att4pool.tile(
                        [128, 4, 512], BF16, tag="att4", name=f"a4_{sb}_{grp}_{jh}"
                    )
            for jh in range(2):
                aps = attpool.tile([128, 512], F32, tag="att", name=f"ap{sb}_{grp}{m}{jh}")
                nc.tensor.matmul(
                    aps[:],
                    avfR[32 * m : 32 * m + 32, 128 * jh : 128 * jh + 128],
                    cvfZ[32 * m : 32 * m + 32, :],
                    start=True,
                    stop=True,
                    tile_position=(32 * m, 0),
                )
                nc.scalar.activation(
                    ctx["att4"][grp][jh][:, m, :], aps[:], AF.Tanh, scale=SCALE
                )

        def emit_ht(ctx, grp, g):
            """H = relu(Wca@att + Wa@fts) for 4 samples (subgroup g)."""
            sb = ctx["b"]
            avff = ctx["avff"][grp]
            for bi in range(2):
                htps = htpool.tile([32, 512], F32, tag="ht", name=f"ht{sb}_{grp}{g}{bi}")
                nc.tensor.matmul(
                    htps[:],
                    wa_sb[:],
                    avff[:, bi, 4 * g : 4 * g + 4, :],
                    start=True,
                    stop=False,
                )
                for jh in range(2):
                    v = ctx["att4"][grp][jh][:].rearrange(
                        "p m (s2 bi c) -> p m s2 bi c", s2=2, bi=2
                    )
                    nc.tensor.matmul(
                        htps[:],
                        wca_sb[:, jh, :],
                        v[:, 2 * g : 2 * g + 2, :, bi, :],
                        start=False,
                        stop=(jh == 1),
                    )
                htsb = htsbpool.tile(
                    [32, 512], BF16, tag="htsb", name=f"hs{sb}_{grp}{g}{bi}"
                )
                nc.vector.tensor_relu(htsb[:], htps[:])
                ctx["htsb"][(grp, g, bi)] = htsb

        def emit_out(ctx, grp, g):
            """out = Wh@H into outp PSUM."""
            outp = ctx["outp"][grp]
            for bi in range(2):
                htsb = ctx["htsb"].pop((grp, g, bi))
                for q in range(4):
                    c0 = 128 * g + 64 * bi + 16 * q
                    nc.tensor.matmul(
                        outp[:, c0 : c0 + 16],
                        htsb[0:32, 128 * q : 128 * q + 128],
                        wh_sb[0:32, 16 * bi : 16 * bi + 16],
                        start=True,
                        stop=True,
                    )

        def emit_adds(ctx, grp):
            """residual adds into outa/outv (bf16 SBUF)."""
            outp = ctx["outp"][grp]
            opv = outp[:].rearrange("p (g bi q t) -> p g bi q t", g=2, bi=2, q=4)
            oa = ctx["outa"][:, 128 * grp : 128 * grp + 128].rearrange(
                "p (g q t) -> p g q t", g=2, q=4
            )
            ov = ctx["outv"][:, 128 * grp : 128 * grp + 128].rearrange(
                "p (g q t) -> p g q t", g=2, q=4
            )
            aT = ctx["audT"][:, 128 * grp : 128 * grp + 128].rearrange(
                "p (g q t) -> p g q t", g=2, q=4
            )
            vT = ctx["visT"][:, 128 * grp : 128 * grp + 128].rearrange(
                "p (g q t) -> p g q t", g=2, q=4
            )
            nc.vector.tensor_add(oa, opv[:, :, 0, :, :], aT)
            nc.vector.tensor_add(ov, opv[:, :, 1, :, :], vT)

        def emit_reg(ctx):
            sb = ctx["b"]
            rb, off = ctx["RB"], ctx["OFF"]
            regps = trpool.tile([2, 256], F32, tag="tr", name=f"reg{sb}")
            nc.tensor.matmul(
                regps[:, 0:rb], wrega_sb[:], ctx["outa"][:], start=True, stop=False
            )
            nc.tensor.matmul(
                regps[:, 0:rb], wregv_sb[:], ctx["outv"][:], start=False, stop=True
            )
            fin = finpool.tile([2, 256], F32, tag="fin", name=f"fin{sb}")
            nc.scalar.activation(fin[:, 0:rb], regps[:, 0:rb], AF.Identity, bias=creg_sb[:])
            nc.sync.dma_start(io["vouts"][off : off + rb], fin[0:1, 0:rb])
            nc.sync.dma_start(io["aouts"][off : off + rb], fin[1:2, 0:rb])

        def make_units(ctx, ngrp):
            # fronts of all groups first: their transpose->copy->avff chains
            # get maximum lead time before the dependent att/ht work
            units = []
            for grp in range(ngrp):
                units.append(lambda grp=grp: emit_front(ctx, grp))
            for grp in range(ngrp):
                units.append(lambda grp=grp: emit_att(ctx, grp, 0))
                units.append(lambda grp=grp: emit_att(ctx, grp, 1))
                units.append(lambda grp=grp: emit_ht(ctx, grp, 0))
                units.append(lambda grp=grp: emit_att(ctx, grp, 2))
                units.append(lambda grp=grp: emit_att(ctx, grp, 3))
                units.append(lambda grp=grp: emit_ht(ctx, grp, 1))
                units.append(lambda grp=grp: emit_out(ctx, grp, 0))
                units.append(lambda grp=grp: emit_out(ctx, grp, 1))
                units.append(lambda grp=grp: emit_adds(ctx, grp))
            units.append(lambda: emit_reg(ctx))
            return units

        prev_units = []
        for b in range(D.NSB):
            RB = D.RB_LIST[b]
            OFF = D.OFF_LIST[b]
            NGRP = D.SBS_LIST[b] // 8
            ctx = {"b": b, "RB": RB, "OFF": OFF, "avfR": {}, "avff": {},
                   "cvfZ": {}, "att4": {}, "htsb": {}}
            # one PSUM bank holds both encoder accumulators for this sub-batch
            encps = encpool.tile([128, 512], F32, tag="enc", name=f"encps{b}")
            audps = encps[:, 0:RB]
            visps = encps[:, 256 : 256 + RB]
            for c in range(4):
                nc.tensor.matmul(
                    audps,
                    wenc1_sb[:, c, :],
                    f1_sb[:, c, OFF : OFF + RB],
                    start=(c == 0),
                    stop=(c == 3),
                )
            audT = actpool.tile([128, 256], BF16, tag="act", name=f"audT{b}")
            nc.scalar.activation(audT[:, 0:RB], audps, AF.Identity, bias=b1_sb[:])
            ctx["audT"] = audT

            # vis stream, interleaving branch work of sub-batch b-1
            emitted = 0
            for i in range(D.NST):
                xg_dram = io[f"xg{b}"][i]
                if b == 0 and i == 0:
                    xg = xg00
                    gh = D.G // 2
                    nc.sync.dma_start(xg[:, gh:, :], xg_dram[:, gh:, :])
                    nc.sync.dma_start(wt_tiles[0][:, gh:, :], io["wt"][0][:, gh:, :])
                else:
                    xg = xpool.tile([128, D.G, RB], XDT, tag="xg", name=f"xg{b}_{i}")
                    nc.sync.dma_start(xg[:], xg_dram)
                    if b == 0:
                        nc.sync.dma_start(wt_tiles[i][:], io["wt"][i])
                        if i == 4:
                            issue_branch_consts()
                        if i == 10:
                            nc.scalar.dma_start(
                                f1_sb[:, :, D.RB0 :], io["f1t"][:, :, D.RB0 :]
                            )
                for j in range(D.G):
                    k = D.G * i + j
                    nc.tensor.matmul(
                        visps,
                        wt_tiles[i][:, j, :],
                        xg[:, j, :],
                        start=(k == 0),
                        stop=(k == D.KC - 1),
                    )
                target = (i + 1) * len(prev_units) // D.NST
                while emitted < target:
                    prev_units[emitted]()
                    emitted += 1
            while emitted < len(prev_units):
                prev_units[emitted]()
                emitted += 1
            visT = actpool.tile([128, 256], BF16, tag="act", name=f"visT{b}")
            nc.scalar.activation(
                visT[:, 0:RB],
                visps,
                AF.Identity,
                bias=b2_sb[:],
                scale=(1.0 / WSCALE) if USE_FP8 else 1.0,
            )
            ctx["visT"] = visT
            outa = outsbpool.tile([128, RB], BF16, tag="outsb", name=f"oa{b}")
            outv = outsbpool.tile([128, RB], BF16, tag="outsb", name=f"ov{b}")
            ctx["outa"], ctx["outv"] = outa, outv
            ctx["outp"] = {}
            for grp in range(NGRP):
                ctx["outp"][grp] = outpool.tile(
                    [128, 256], F32, tag="outp", name=f"op{b}_{grp}"
                )
            prev_units = make_units(ctx, NGRP)

        for u in prev_units:
            u()


def _build(D: Dims):
    nc = bacc.Bacc("TRN2", target_bir_lowering=False, debug=False)

    io = {}
    for b in range(D.NSB):
        io[f"xg{b}"] = nc.dram_tensor(
            f"xg{b}", [D.NST, 128, D.G, D.RB_LIST[b]], XDT, kind="ExternalInput"
        ).ap()
    io["wt"] = nc.dram_tensor(
        "wt", [D.NST, 128, D.G, 128], XDT, kind="ExternalInput"
    ).ap()
    io["f1t"] = nc.dram_tensor("f1t", [128, 4, D.R], BF16, kind="ExternalInput").ap()
    io["wenc1"] = nc.dram_tensor("wenc1", [128, 4, DH], BF16, kind="ExternalInput").ap()
    for name, shape, dt in [
        ("b1", [DH, 1], F32),
        ("b2", [DH, 1], F32),
        ("wblk4", [128, 4, 128], BF16),
        ("waT", [16, 32], BF16),
        ("wcaT", [128, 2, 32], BF16),
        ("whT", [32, 32], BF16),
        ("wreg_a", [128, 2], BF16),
        ("wreg_v", [128, 2], BF16),
        ("creg", [2, 1], F32),
        ("ident", [128, 128], BF16),
    ]:
        io[name] = nc.dram_tensor(name, shape, dt, kind="ExternalInput").ap()
    io["vouts"] = nc.dram_tensor("vouts", [D.R], F32, kind="ExternalOutput").ap()
    io["aouts"] = nc.dram_tensor("aouts", [D.R], F32, kind="ExternalOutput").ap()

    with tile.TileContext(nc) as tc:
        build_graph(tc, io, D)

    nc.compile()
    return nc


def prep_shared(inputs, D: Dims):
    f32 = np.float32
    bf = ml_dtypes.bfloat16
    W_enc1 = np.asarray(inputs["W_enc1"], f32)
    W_enc2 = np.asarray(inputs["W_enc2"], f32)
    W_red = np.asarray(inputs["W_red"], f32)
    W2r = W_enc2 @ W_red                                    # [128, DV]
    b2v = W_enc2 @ np.asarray(inputs["b_red"], f32) + np.asarray(inputs["b_enc2"], f32)
    wv = (np.asarray(inputs["Wv2"], f32) @ np.asarray(inputs["Wv1"], f32))[0]
    cv = float((np.asarray(inputs["Wv2"], f32) @ np.asarray(inputs["bv1"], f32)
                + np.asarray(inputs["bv2"], f32))[0])
    wa = (np.asarray(inputs["Wa2"], f32) @ np.asarray(inputs["Wa1"], f32))[0]
    ca = float((np.asarray(inputs["Wa2"], f32) @ np.asarray(inputs["ba1"], f32)
                + np.asarray(inputs["ba2"], f32))[0])

    def mk_wblk(W_aff, odd):
        # lhsT[16s+t', 32m+16*odd+t] = W_aff[t, t'] for s = 2m+odd
        M = np.zeros((128, 128), f32)
        Wt = np.asarray(W_aff, f32).T  # [t', t]
        for s in range(odd, 8, 2):
            m = s // 2
            base = 32 * m + 16 * odd
            M[16 * s : 16 * s + 16, base : base + 16] = Wt
        return M

    wblk4 = np.stack(
        [
            mk_wblk(inputs["W_affa"], 0),
            mk_wblk(inputs["W_affv"], 0),
            mk_wblk(inputs["W_affa"], 1),
            mk_wblk(inputs["W_affv"], 1),
        ],
        axis=1,
    )  # [128, 4, 128]

    wt = (
        np.ascontiguousarray(W2r.T)
        .reshape(D.NST, D.G, 128, 128)
        .transpose(0, 2, 1, 3)
    )
    if USE_FP8:
        wt = wt * WSCALE
    wenc1t = W_enc1.T.reshape(4, 128, DH).transpose(1, 0, 2)

    wh = np.zeros((32, 32), f32)
    wh[:, 0:16] = np.asarray(inputs["W_ha"], f32).T
    wh[:, 16:32] = np.asarray(inputs["W_hv"], f32).T

    shared = {
        "wt": np.ascontiguousarray(wt).astype(
            ml_dtypes.float8_e3m4 if USE_FP8 else bf
        ),
        "wenc1": np.ascontiguousarray(wenc1t).astype(bf),
        "b1": np.asarray(inputs["b_enc1"], f32).reshape(DH, 1),
        "b2": b2v.reshape(DH, 1),
        "wblk4": np.ascontiguousarray(wblk4).astype(bf),
        "waT": np.asarray(inputs["W_a"], f32).T.astype(bf),
        "wcaT": np.asarray(inputs["W_ca"], f32)
        .T.reshape(2, 128, 32)
        .transpose(1, 0, 2)
        .astype(bf)
        .copy(),
        "whT": wh.astype(bf),
        "wreg_a": np.stack([wv[:128], wa[:128]], 1).astype(bf),
        "wreg_v": np.stack([wv[128:], wa[128:]], 1).astype(bf),
        "creg": np.array([[cv], [ca]], f32),
        "ident": np.eye(128, dtype=f32).astype(bf),
    }
    return shared


def prep_core(f1_core, f2_core, D: Dims):
    """Per-core activation tiling. f1_core [R, DA], f2_core [R, DV] fp32."""
    bf = ml_dtypes.bfloat16
    xdt = ml_dtypes.float8_e3m4 if USE_FP8 else bf
    out = {}
    for b in range(D.NSB):
        rb, off = D.RB_LIST[b], D.OFF_LIST[b]
        # xg_b[i, p, j, r] = f2[off + r, (G*i+j)*128 + p]
        xg = (
            f2_core[off : off + rb]
            .reshape(rb, D.NST, D.G, 128)
            .transpose(1, 3, 2, 0)
        )
        out[f"xg{b}"] = np.ascontiguousarray(xg).astype(xdt)
    # f1t[p, c, r] = f1[r, c*128+p]
    f1t = f1_core.reshape(D.R, 4, 128).transpose(2, 1, 0)
    out["f1t"] = np.ascontiguousarray(f1t).astype(bf)
    return out


def kernel(**inputs):
    D = Dims(DV=DVFULL, S=B // NCORES)
    if "nc" not in _CACHE:
        _CACHE["nc"] = _build(D)
    nc = _CACHE["nc"]

    shared = prep_shared(inputs, D)
    f1 = np.asarray(inputs["f1_norm"], np.float32).reshape(B * T, DA)
    f2 = np.asarray(inputs["f2_norm"], np.float32).reshape(B * T, DVFULL)

    in_maps = []
    for c in range(NCORES):
        rs = slice(c * D.R, (c + 1) * D.R)
        m = dict(shared)
        m.update(prep_core(f1[rs], f2[rs], D))
        in_maps.append(m)

    import os

    res = bass_utils.run_bass_kernel_spmd(
        nc,
        in_maps,
        core_ids=list(range(NCORES)),
        trace=bool(os.environ.get("KERNEL_TRACE")),
    )
    _CACHE["last_results"] = res

    S = B // NCORES
    vouts = np.concatenate(
        [r["vouts"].reshape(S, T) for r in res.results], axis=0
    ).astype(np.float32)
    aouts = np.concatenate(
        [r["aouts"].reshape(S, T) for r in res.results], axis=0
    ).astype(np.float32)
    return vouts, aouts
